# revision 1
# baseline (speedup 1.0000x reference)
"""MCR2 loss kernel for 8 Trainium2 NeuronCores.

Data-parallel over the sample axis: each core streams its 75000-row shard
of Z once, building per-128-sample-tile one-hot-masked copies of Z with a
single fused DVE scalar_tensor_tensor op (M[p, j*32+f] = (j == label_p) *
Z[p, f]) and accumulating Z_tile^T @ M_tile into PSUM, which yields all 10
per-class Grams Gj = Z^T diag(Pi_j) Z.  G = sum_j Gj exactly (one-hot).
The tiny [10,32,32] partials are summed on the host and the 32x32 logdets
are evaluated there in float64.
"""

import os
import sys

sys.path.insert(0, "/opt/trn_rl_repo")

import numpy as np

import concourse.bacc as bacc
import concourse.bass as bass
import concourse.mybir as mybir
import concourse.tile as tile
from concourse.bass_utils import run_bass_kernel_spmd

N, D, C = 600000, 32, 10
EPS = 0.5
NCORES = 8
PER = N // NCORES            # 75000 rows per core
PAD = ((PER + 127) // 128) * 128   # 75008
NTILES = PAD // 128          # 586 tiles of 128 samples
T_FULL = 32                  # tiles per chunk
FULL_CHUNKS = NTILES // T_FULL      # 9
T_TAIL = NTILES - FULL_CHUNKS * T_FULL  # 10
MW = C * D                   # 320: masked block width

_cache = {}


def _build_program():
    nc = bacc.Bacc(None)
    z_dram = nc.dram_tensor("Z", [PAD, D], mybir.dt.float32, kind="ExternalInput")
    lab_dram = nc.dram_tensor("labels", [PAD], mybir.dt.int32, kind="ExternalInput")
    out_dram = nc.dram_tensor("grams", [128, MW], mybir.dt.float32, kind="ExternalOutput")

    # class-index constant, value j repeated D times: [128, 320] bf16
    iota_np = np.tile(np.repeat(np.arange(C), D)[None, :], (128, 1)).astype(
        np.dtype("bfloat16") if hasattr(np, "bfloat16") else np.float32
    )
    # ml_dtypes bfloat16 via mybir numpy mapping
    import ml_dtypes

    iota_np = np.tile(np.arange(C)[None, :], (128, 1)).astype(ml_dtypes.bfloat16)
    iota_dram = nc.inline_tensor(iota_np, name="iota_c")

    bf16 = mybir.dt.bfloat16
    f32 = mybir.dt.float32

    with tile.TileContext(nc) as tc:
        with (
            tc.tile_pool(name="zraw", bufs=2) as zraw_pool,
            tc.tile_pool(name="zin", bufs=2) as zin_pool,
            tc.tile_pool(name="lab", bufs=2) as lab_pool,
            tc.tile_pool(name="labf", bufs=2) as labf_pool,
            tc.tile_pool(name="mask", bufs=2) as m_pool,
            tc.tile_pool(name="mask10", bufs=2) as mk_pool,
            tc.tile_pool(name="const", bufs=1) as const_pool,
            tc.tile_pool(name="outp", bufs=1) as out_pool,
            tc.tile_pool(name="psum", bufs=1, space="PSUM") as psum_pool,
        ):
            iota_sb = const_pool.tile([128, C], bf16)
            nc.sync.dma_start(iota_sb[:], iota_dram[:])
            # Tiny DVE read of the const so the DVE engine's vector clock
            # observes the const DMA once, instead of the wait landing on a
            # later STT (walrus: "Too many sync wait commands").
            touch = const_pool.tile([128, 2], bf16)
            nc.vector.tensor_copy(touch[:], iota_sb[:, 0:2])

            acc = psum_pool.tile([128, MW], f32)

            z_full = z_dram[0 : FULL_CHUNKS * 128 * T_FULL, :].rearrange(
                "(c p t) d -> c p (t d)", p=128, t=T_FULL
            )
            lab_full = lab_dram[0 : FULL_CHUNKS * 128 * T_FULL].rearrange(
                "(c p t) -> c p t", p=128, t=T_FULL
            )
            z_tail = z_dram[FULL_CHUNKS * 128 * T_FULL :, :].rearrange(
                "(p t) d -> p (t d)", p=128, t=T_TAIL
            )
            lab_tail = lab_dram[FULL_CHUNKS * 128 * T_FULL :].rearrange(
                "(p t) -> p t", p=128, t=T_TAIL
            )

            gtile = 0
            for c in range(FULL_CHUNKS + 1):
                tchunk = T_FULL if c < FULL_CHUNKS else T_TAIL
                z_raw = zraw_pool.tile([128, T_FULL * D], f32, tag="zr")
                z_sb = zin_pool.tile([128, T_FULL * D], bf16, tag="z")
                lab_sb = lab_pool.tile([128, T_FULL], mybir.dt.int32, tag="l")
                labf_sb = labf_pool.tile([128, T_FULL], bf16, tag="lf")
                if c < FULL_CHUNKS:
                    nc.sync.dma_start(z_raw[:, : tchunk * D], z_full[c])
                    nc.sync.dma_start(lab_sb[:, :tchunk], lab_full[c])
                else:
                    nc.sync.dma_start(z_raw[:, : tchunk * D], z_tail[:])
                    nc.sync.dma_start(lab_sb[:, :tchunk], lab_tail[:])
                nc.vector.tensor_copy(labf_sb[:, :tchunk], lab_sb[:, :tchunk])
                # fp32 -> bf16 cast on the otherwise-idle Scalar engine; also
                # the single sync point between the Z DMA and downstream readers.
                nc.scalar.mul(z_sb[:, : tchunk * D], z_raw[:, : tchunk * D], 1.0)

                # one-hot mask for the whole chunk: [128, t, j]
                mk_sb = mk_pool.tile([128, T_FULL * C], bf16, tag="mk")
                nc.vector.tensor_tensor(
                    out=mk_sb[:, : tchunk * C].rearrange("p (t j) -> p t j", j=C),
                    in0=labf_sb[:, :tchunk].unsqueeze(2).broadcast_to(
                        [128, tchunk, C]
                    ),
                    in1=iota_sb[:].unsqueeze(1).broadcast_to([128, tchunk, C]),
                    op=mybir.AluOpType.is_equal,
                )
                # masked copies for the whole chunk in one wide multiply:
                # M[p, t, j, f] = mask[p, t, j] * Z[p, t, f]
                m_sb = m_pool.tile([128, T_FULL * MW], bf16, tag="m")
                for eng, lo, hi in ((nc.vector, 0, tchunk),):
                    nt = hi - lo
                    eng.tensor_tensor(
                        out=m_sb[:, lo * MW : hi * MW].rearrange(
                            "p (t j f) -> p t j f", j=C, f=D
                        ),
                        in0=mk_sb[:, lo * C : hi * C]
                        .rearrange("p (t j) -> p t j", j=C)
                        .unsqueeze(3)
                        .broadcast_to([128, nt, C, D]),
                        in1=z_sb[:, lo * D : hi * D]
                        .rearrange("p (t f) -> p t f", f=D)
                        .unsqueeze(2)
                        .broadcast_to([128, nt, C, D]),
                        op=mybir.AluOpType.mult,
                    )
                for t in range(tchunk):
                    grp = gtile % 4
                    nc.tensor.matmul(
                        acc[grp * D : (grp + 1) * D, :],
                        z_sb[:, t * D : (t + 1) * D],
                        m_sb[:, t * MW : (t + 1) * MW],
                        start=(gtile < 4),
                        stop=(gtile >= NTILES - 4),
                        tile_position=(0, grp * D),
                    )
                    gtile += 1

            out_sb = out_pool.tile([128, MW], f32)
            nc.vector.tensor_copy(out_sb[:], acc[:])
            nc.sync.dma_start(out_dram[:], out_sb[:])

    nc.compile()
    return nc


def kernel(Z: np.ndarray, labels: np.ndarray) -> np.ndarray:
    Z = np.asarray(Z, dtype=np.float32)
    labels = np.asarray(labels, dtype=np.int32)

    if "nc" not in _cache:
        _cache["nc"] = _build_program()
    nc = _cache["nc"]

    in_maps = []
    for k in range(NCORES):
        zs = Z[k * PER : (k + 1) * PER]
        ls = labels[k * PER : (k + 1) * PER]
        zp = np.zeros([PAD, D], np.float32)
        zp[:PER] = zs
        lp = np.zeros([PAD], np.int32)
        lp[:PER] = ls
        in_maps.append({"Z": zp, "labels": lp})

    res = run_bass_kernel_spmd(nc, in_maps, core_ids=list(range(NCORES)))
    _cache["last_results"] = res

    gj = np.zeros([C, D, D], np.float64)
    for r in res.results:
        g = r["grams"].astype(np.float64).reshape(4, D, MW).sum(axis=0)
        for j in range(C):
            gj[j] += g[:, j * D : (j + 1) * D]

    g_all = gj.sum(axis=0)
    tr_pi = np.bincount(labels, minlength=C).astype(np.float64)

    nf, df = float(N), float(D)
    eye = np.eye(D)
    loss_r = 0.5 * np.linalg.slogdet(eye + (df / (nf * EPS)) * g_all)[1]
    loss_rc = 0.0
    for j in range(C):
        ld = np.linalg.slogdet(eye + (df / (tr_pi[j] * EPS)) * gj[j])[1]
        loss_rc += (tr_pi[j] / (2.0 * nf)) * ld
    loss_obj = loss_r - loss_rc
    return np.asarray([-loss_obj, loss_r, loss_rc], dtype=np.float32)



# revision 2
# speedup vs baseline: 5.5205x; 5.5205x over previous
"""MCR2 loss kernel for 8 Trainium2 NeuronCores.

Data-parallel over the sample axis.  The host permutes each core's
75000-row shard of Z so rows are grouped by class (each class padded with
zero rows to a fixed capacity CAP, a multiple of 512) and lays the rows
out in the exact SBUF tile layout the device wants:

    Zb[p, t*32 + f] = rows[t*128 + p, f]      (bf16)

On device each group of 4 consecutive 128-row tiles forms X = [128, 128]
(4 tiles side by side); one full-width matmul X^T @ X accumulated in PSUM
yields all four tile-Grams on its 32x32 diagonal blocks.  16 such matmuls
per class produce the per-class Gram partials with zero per-sample work on
the Vector/Scalar engines.  The tiny [128, 10*128] PSUM results are copied
to SBUF, DMA'd out, and the host sums diagonal blocks across cores and
evaluates the 32x32 logdets in float64.
"""

import sys

sys.path.insert(0, "/opt/trn_rl_repo")

import ml_dtypes
import numpy as np

import concourse.bacc as bacc
import concourse.mybir as mybir
import concourse.tile as tile
from concourse.bass_utils import run_bass_kernel_spmd

N, D, C = 600000, 32, 10
EPS = 0.5
NCORES = 8
PER = N // NCORES            # 75000 rows per core
GROUP_ROWS = 512             # 4 tiles of 128 rows -> one [128,128] matmul

_cache = {}


def _build_program(cap):
    """cap: per-class row capacity (multiple of 512)."""
    gpc = cap // GROUP_ROWS          # matmul groups per class
    tiles = C * cap // 128           # 128-row tiles total
    ncols = tiles * 32               # SBUF columns of Zb

    bf16 = mybir.dt.bfloat16
    f32 = mybir.dt.float32

    nc = bacc.Bacc(None)
    z_dram = nc.dram_tensor("Zb", [128, ncols], bf16, kind="ExternalInput")
    out_dram = nc.dram_tensor("grams", [128, C * 128], f32, kind="ExternalOutput")

    with tile.TileContext(nc) as tc:
        with (
            tc.tile_pool(name="zin", bufs=2 * C) as zin_pool,
            tc.tile_pool(name="outp", bufs=4) as out_pool,
            tc.tile_pool(name="psum", bufs=4, space="PSUM") as psum_pool,
        ):
            for j in range(C):
                acc = psum_pool.tile([128, 128], f32, tag="acc")
                # two DMA chunks per class so the PE can start on the first
                # half while the second streams in
                half = gpc // 2
                parts = [(0, half), (half, gpc)]
                for g0, g1 in parts:
                    ng = g1 - g0
                    z_sb = zin_pool.tile([128, 8 * 128], bf16, tag="z")
                    c0 = (j * gpc + g0) * 128
                    nc.sync.dma_start(
                        z_sb[:, : ng * 128], z_dram[:, c0 : c0 + ng * 128]
                    )
                    for g in range(ng):
                        w = z_sb[:, g * 128 : (g + 1) * 128]
                        nc.tensor.matmul(
                            acc[:],
                            w,
                            w,
                            start=(g0 + g == 0),
                            stop=(g0 + g == gpc - 1),
                        )
                out_sb = out_pool.tile([128, 128], f32, tag="o")
                nc.vector.tensor_copy(out_sb[:], acc[:])
                nc.sync.dma_start(out_dram[:, j * 128 : (j + 1) * 128], out_sb[:])

    nc.compile()
    return nc


def kernel(Z: np.ndarray, labels: np.ndarray) -> np.ndarray:
    Z = np.asarray(Z, dtype=np.float32)
    labels = np.asarray(labels, dtype=np.int32)

    # per-core class counts decide the (compile-time) class capacity
    counts = np.stack(
        [
            np.bincount(labels[k * PER : (k + 1) * PER], minlength=C)
            for k in range(NCORES)
        ]
    )
    cap = int(np.ceil(counts.max() / GROUP_ROWS) * GROUP_ROWS)
    key = ("nc", cap)
    if key not in _cache:
        _cache[key] = _build_program(cap)
    nc = _cache[key]

    tiles = C * cap // 128
    in_maps = []
    for k in range(NCORES):
        zs = Z[k * PER : (k + 1) * PER]
        ls = labels[k * PER : (k + 1) * PER]
        order = np.argsort(ls, kind="stable")
        srt = zs[order]
        buf = np.zeros([C * cap, D], dtype=ml_dtypes.bfloat16)
        off = 0
        for j in range(C):
            cnt = int(counts[k, j])
            buf[j * cap : j * cap + cnt] = srt[off : off + cnt]
            off += cnt
        zb = np.ascontiguousarray(
            buf.reshape(tiles, 128, D).transpose(1, 0, 2).reshape(128, tiles * D)
        )
        in_maps.append({"Zb": zb})

    res = run_bass_kernel_spmd(nc, in_maps, core_ids=list(range(NCORES)))
    _cache["last_results"] = res

    # host: sum the 4 diagonal 32x32 blocks per class across cores
    gj = np.zeros([C, D, D], np.float64)
    for r in res.results:
        g = r["grams"].astype(np.float64)  # [128, C*128]
        for j in range(C):
            blk = g[:, j * 128 : (j + 1) * 128]
            for a in range(4):
                gj[j] += blk[a * D : (a + 1) * D, a * D : (a + 1) * D]

    g_all = gj.sum(axis=0)
    tr_pi = np.bincount(labels, minlength=C).astype(np.float64)

    nf, df = float(N), float(D)
    eye = np.eye(D)
    loss_r = 0.5 * np.linalg.slogdet(eye + (df / (nf * EPS)) * g_all)[1]
    loss_rc = 0.0
    for j in range(C):
        ld = np.linalg.slogdet(eye + (df / (tr_pi[j] * EPS)) * gj[j])[1]
        loss_rc += (tr_pi[j] / (2.0 * nf)) * ld
    loss_obj = loss_r - loss_rc
    return np.asarray([-loss_obj, loss_r, loss_rc], dtype=np.float32)


# revision 3
# speedup vs baseline: 7.2799x; 1.3187x over previous
"""MCR2 loss kernel for 8 Trainium2 NeuronCores.

Data-parallel over the sample axis.  The host permutes each core's
75000-row shard of Z so rows are grouped by class (each class padded with
zero rows to a fixed capacity CAP, a multiple of 512) and lays the rows
out in the exact SBUF tile layout the device wants:

    Zb[p, t*32 + f] = rows[t*128 + p, f]      (bf16)

On device each group of 4 consecutive 128-row tiles forms X = [128, 128]
(4 tiles side by side); one full-width matmul X^T @ X accumulated in PSUM
yields all four tile-Grams on its 32x32 diagonal blocks.  16 such matmuls
per class produce the per-class Gram partials with zero per-sample work on
the Vector/Scalar engines.  The tiny [128, 10*128] PSUM results are copied
to SBUF, DMA'd out, and the host sums diagonal blocks across cores and
evaluates the 32x32 logdets in float64.
"""

import sys

sys.path.insert(0, "/opt/trn_rl_repo")

import ml_dtypes
import numpy as np

import concourse.bacc as bacc
import concourse.mybir as mybir
import concourse.tile as tile
from concourse.bass_utils import run_bass_kernel_spmd

N, D, C = 600000, 32, 10
EPS = 0.5
NCORES = 8
PER = N // NCORES            # 75000 rows per core
GROUP_ROWS = 512             # 4 tiles of 128 rows -> one [128,128] matmul

_cache = {}


def _build_program(cap):
    """cap: per-class row capacity (multiple of 512)."""
    gpc = cap // GROUP_ROWS          # matmul groups per class
    tiles = C * cap // 128           # 128-row tiles total
    ncols = tiles * 32               # SBUF columns of Zb

    bf16 = mybir.dt.bfloat16
    f32 = mybir.dt.float32

    nc = bacc.Bacc(None)
    z_dram = nc.dram_tensor("Zb", [128, ncols], bf16, kind="ExternalInput")
    out_dram = nc.dram_tensor("grams", [128, C * 128], f32, kind="ExternalOutput")

    with tile.TileContext(nc) as tc:
        with (
            tc.tile_pool(name="zin", bufs=C) as zin_pool,
            tc.tile_pool(name="outp", bufs=4) as out_pool,
            tc.tile_pool(name="psum", bufs=4, space="PSUM") as psum_pool,
        ):
            for j in range(C):
                acc = psum_pool.tile([128, 128], f32, tag="acc")
                z_sb = zin_pool.tile([128, gpc * 128], bf16, tag="z")
                # input DMAs on the SP queue only: they must never sit
                # behind a post that waits on compute
                nc.sync.dma_start(z_sb[:], z_dram[:, j * gpc * 128 : (j + 1) * gpc * 128])
                for g in range(gpc):
                    w = z_sb[:, g * 128 : (g + 1) * 128]
                    nc.tensor.matmul(
                        acc[:],
                        w,
                        w,
                        start=(g == 0),
                        stop=(g == gpc - 1),
                    )
                out_sb = out_pool.tile([128, 128], f32, tag="o")
                nc.vector.tensor_copy(out_sb[:], acc[:])
                # output DMAs posted from the Activation engine (also HWDGE)
                # so their copy-dependent waits don't stall SP input posting
                nc.scalar.dma_start(out_dram[:, j * 128 : (j + 1) * 128], out_sb[:])

    nc.compile()
    return nc


def kernel(Z: np.ndarray, labels: np.ndarray) -> np.ndarray:
    Z = np.asarray(Z, dtype=np.float32)
    labels = np.asarray(labels, dtype=np.int32)

    # per-core class counts decide the (compile-time) class capacity
    counts = np.stack(
        [
            np.bincount(labels[k * PER : (k + 1) * PER], minlength=C)
            for k in range(NCORES)
        ]
    )
    cap = int(np.ceil(counts.max() / GROUP_ROWS) * GROUP_ROWS)
    key = ("nc", cap)
    if key not in _cache:
        _cache[key] = _build_program(cap)
    nc = _cache[key]

    tiles = C * cap // 128
    in_maps = []
    for k in range(NCORES):
        zs = Z[k * PER : (k + 1) * PER]
        ls = labels[k * PER : (k + 1) * PER]
        order = np.argsort(ls, kind="stable")
        srt = zs[order]
        buf = np.zeros([C * cap, D], dtype=ml_dtypes.bfloat16)
        off = 0
        for j in range(C):
            cnt = int(counts[k, j])
            buf[j * cap : j * cap + cnt] = srt[off : off + cnt]
            off += cnt
        zb = np.ascontiguousarray(
            buf.reshape(tiles, 128, D).transpose(1, 0, 2).reshape(128, tiles * D)
        )
        in_maps.append({"Zb": zb})

    res = run_bass_kernel_spmd(nc, in_maps, core_ids=list(range(NCORES)))
    _cache["last_results"] = res

    # host: sum the 4 diagonal 32x32 blocks per class across cores
    gj = np.zeros([C, D, D], np.float64)
    for r in res.results:
        g = r["grams"].astype(np.float64)  # [128, C*128]
        for j in range(C):
            blk = g[:, j * 128 : (j + 1) * 128]
            for a in range(4):
                gj[j] += blk[a * D : (a + 1) * D, a * D : (a + 1) * D]

    g_all = gj.sum(axis=0)
    tr_pi = np.bincount(labels, minlength=C).astype(np.float64)

    nf, df = float(N), float(D)
    eye = np.eye(D)
    loss_r = 0.5 * np.linalg.slogdet(eye + (df / (nf * EPS)) * g_all)[1]
    loss_rc = 0.0
    for j in range(C):
        ld = np.linalg.slogdet(eye + (df / (tr_pi[j] * EPS)) * gj[j])[1]
        loss_rc += (tr_pi[j] / (2.0 * nf)) * ld
    loss_obj = loss_r - loss_rc
    return np.asarray([-loss_obj, loss_r, loss_rc], dtype=np.float32)


# revision 5
# speedup vs baseline: 8.5326x; 1.1721x over previous
"""MCR2 loss kernel for 8 Trainium2 NeuronCores.

Data-parallel over the sample axis.  The host permutes each core's
75000-row shard of Z so rows are grouped by class (each class padded with
zero rows to a fixed capacity CAP, a multiple of 1024), quantizes to fp8
e4m3, and lays the rows out in the exact SBUF tile layout the device
wants:

    Zb[p, t*32 + f] = rows[t*128 + p, f]

On device, each group of 4 consecutive 128-row tiles forms X = [128, 128]
(4 tiles side by side); one full-width fp8 matmul X^T @ X accumulated in
PSUM yields all four tile-Grams on its 32x32 diagonal blocks (fp8 streams
at bf16 PE speed and the 128-col weights get automatic fast-weight-load).
16 such matmuls per class give
the per-class Gram partial with zero per-sample Vector/Scalar work.  Input
DMAs are posted from SP, output DMAs from the Activation engine so
compute-dependent output posts never stall input streaming.  The host
sums diagonal blocks across cores and evaluates the logdets in float64.
"""

import sys

sys.path.insert(0, "/opt/trn_rl_repo")

import ml_dtypes
import numpy as np

import concourse.bacc as bacc
import concourse.mybir as mybir
import concourse.tile as tile
from concourse.bass_utils import run_bass_kernel_spmd

N, D, C = 600000, 32, 10
EPS = 0.5
NCORES = 8
PER = N // NCORES            # 75000 rows per core
GROUP_ROWS = 512             # 4 tiles of 128 rows -> one [128,128] matmul

_cache = {}


def _build_program(cap):
    """cap: per-class row capacity (multiple of 1024)."""
    gpc = cap // GROUP_ROWS          # matmul groups per class
    tiles = C * cap // 128           # 128-row tiles total
    ncols = tiles * 32               # fp8 bytes per partition of Zb

    fp8 = mybir.dt.float8e4
    f32 = mybir.dt.float32

    nc = bacc.Bacc(None)
    z_dram = nc.dram_tensor("Zb", [128, ncols], fp8, kind="ExternalInput")
    out_dram = nc.dram_tensor("grams", [128, C * 128], f32, kind="ExternalOutput")

    with tile.TileContext(nc) as tc:
        with (
            tc.tile_pool(name="zin", bufs=C) as zin_pool,
            tc.tile_pool(name="outp", bufs=4) as out_pool,
            tc.tile_pool(name="psum", bufs=4, space="PSUM") as psum_pool,
        ):
            for j in range(C):
                acc = psum_pool.tile([128, 128], f32, tag="acc")
                z_sb = zin_pool.tile([128, gpc * 128], fp8, tag="z")
                # input DMAs on the SP queue only: they must never sit
                # behind a post that waits on compute
                nc.sync.dma_start(
                    z_sb[:], z_dram[:, j * gpc * 128 : (j + 1) * gpc * 128]
                )
                for g in range(gpc):
                    w = z_sb[:, g * 128 : (g + 1) * 128]
                    nc.tensor.matmul(
                        acc[:],
                        w,
                        w,
                        start=(g == 0),
                        stop=(g == gpc - 1),
                    )
                out_sb = out_pool.tile([128, 128], f32, tag="o")
                nc.vector.tensor_copy(out_sb[:], acc[:])
                # output DMAs posted from the Activation engine (also HWDGE)
                # so their copy-dependent waits don't stall SP input posting
                nc.scalar.dma_start(out_dram[:, j * 128 : (j + 1) * 128], out_sb[:])

    nc.compile()
    return nc


def kernel(Z: np.ndarray, labels: np.ndarray) -> np.ndarray:
    Z = np.asarray(Z, dtype=np.float32)
    labels = np.asarray(labels, dtype=np.int32)

    # per-core class counts decide the (compile-time) class capacity
    counts = np.stack(
        [
            np.bincount(labels[k * PER : (k + 1) * PER], minlength=C)
            for k in range(NCORES)
        ]
    )
    cap = int(np.ceil(counts.max() / GROUP_ROWS) * GROUP_ROWS)
    key = ("nc", cap)
    if key not in _cache:
        _cache[key] = _build_program(cap)
    nc = _cache[key]

    tiles = C * cap // 128
    in_maps = []
    for k in range(NCORES):
        zs = Z[k * PER : (k + 1) * PER]
        ls = labels[k * PER : (k + 1) * PER]
        order = np.argsort(ls, kind="stable")
        srt = zs[order]
        buf = np.zeros([C * cap, D], dtype=ml_dtypes.float8_e4m3)
        off = 0
        for j in range(C):
            cnt = int(counts[k, j])
            buf[j * cap : j * cap + cnt] = srt[off : off + cnt]
            off += cnt
        zb = np.ascontiguousarray(
            buf.reshape(tiles, 128, D).transpose(1, 0, 2).reshape(128, tiles * D)
        )
        in_maps.append({"Zb": zb})

    res = run_bass_kernel_spmd(nc, in_maps, core_ids=list(range(NCORES)))
    _cache["last_results"] = res

    # host: sum the 4 diagonal 32x32 blocks per class across cores
    gj = np.zeros([C, D, D], np.float64)
    for r in res.results:
        g = r["grams"].astype(np.float64)  # [128, C*128]
        for j in range(C):
            blk = g[:, j * 128 : (j + 1) * 128]
            for a in range(4):
                gj[j] += blk[a * D : (a + 1) * D, a * D : (a + 1) * D]

    g_all = gj.sum(axis=0)
    tr_pi = np.bincount(labels, minlength=C).astype(np.float64)

    nf, df = float(N), float(D)
    eye = np.eye(D)
    loss_r = 0.5 * np.linalg.slogdet(eye + (df / (nf * EPS)) * g_all)[1]
    loss_rc = 0.0
    for j in range(C):
        ld = np.linalg.slogdet(eye + (df / (tr_pi[j] * EPS)) * gj[j])[1]
        loss_rc += (tr_pi[j] / (2.0 * nf)) * ld
    loss_obj = loss_r - loss_rc
    return np.asarray([-loss_obj, loss_r, loss_rc], dtype=np.float32)
